# revision 57
# baseline (speedup 1.0000x reference)
"""Transformer block (LN -> causal MHA -> LN -> MLP, residuals) on 8 trn2 NeuronCores.

Data-parallel over batch: each core runs one [T, C] sequence independently
(no collectives). The attention-branch projections (qk, v, Wo) run as
fp8e4 DoubleRow matmuls (2 MACs/cell/cycle, contraction 256 per pass);
layernorm, softmax, residuals and the MLP stay fp32/bf16 — fp8 on the MLP
would push rel-err past the 2e-2 gate, fp8 on the attention branch costs
~1e-3 because softmax output is a near-uniform average (y is ~3% the scale
of the residual stream).

Host-side preprocessing folds the layernorm affine params into the adjacent
matmul weights, folds the V bias through Wo, pre-interleaves the fp8 weight
pairs ([K,2,*] DoubleRow layout), and scales Wqkv by 32 (compensated in the
PSUM epilogues) to keep fp8 weights in e4m3 normal range.
"""

import math
import sys

for _p in ("/opt/trn_rl_repo", "/root/.axon_site/_ro/trn_rl_repo"):
    if _p not in sys.path:
        sys.path.append(_p)

import numpy as np
import ml_dtypes

import concourse.bass as bass
import concourse.mybir as mybir
import concourse.tile as tile
from concourse import bacc
from concourse.bass_utils import run_bass_kernel_spmd

B, T, C, H = 8, 1024, 1024, 16
D = C // H
NT = T // 128          # token tiles
NCK = C // 128         # contraction chunks over C
NCP = NCK // 2         # fp8 DoubleRow chunk-pairs over C
NCK8 = NCK - 2         # bf16 FC-contraction chunks (last pair is fp8)
KN8 = NCK              # bf16 Wp-contraction chunks (last 3/4 is fp8)
F32 = mybir.dt.float32
BF16 = mybir.dt.bfloat16
F8 = mybir.dt.float8e4
DR = mybir.MatmulPerfMode.DoubleRow
AF = mybir.ActivationFunctionType
N_CORES = 8
WS = 32.0              # fp8 weight scale for qk/v (undone in epilogues)


def _pieces(lo, hi, bound=512):
    """Split [lo, hi) at multiples of `bound` (PSUM bank boundaries)."""
    out = []
    a = lo
    while a < hi:
        b = min(hi, (a // bound + 1) * bound)
        out.append((a, b))
        a = b
    return out


def _ln_tile(nc, pools, src, dest, ident_sb, eps_sb, i):
    """One token-tile of layernorm: src [128, NT, C] f32 token-major ->
    feature-major chunk-pairs at dest(cp, i) -> [128, 2, 128] AP (the ACT
    drain converts to the destination dtype, bf16 or fp8).

    Transposes run in bf16 (hw rejects plain fp8 PE-transposes)."""
    stat_pool, htok_pool, psT = pools
    if True:
        xt = src[:, i, :]
        stats = stat_pool.tile([128, 2, 6], F32, tag="lnstats", name="lnstats")
        nc.vector.bn_stats(stats[:, 0, :], xt[:, 0:512])
        nc.vector.bn_stats(stats[:, 1, :], xt[:, 512:1024])
        mv = stat_pool.tile([128, 2], F32, tag="lnmv", name="lnmv")
        nc.vector.bn_aggr(mv, stats)
        std = stat_pool.tile([128, 1], F32, tag="lnstd", name="lnstd")
        nc.scalar.activation(std, mv[:, 1:2], AF.Sqrt, bias=eps_sb, scale=1.0)
        rstd = stat_pool.tile([128, 1], F32, tag="lnrstd", name="lnrstd")
        nc.vector.reciprocal(rstd, std)
        ht = htok_pool.tile([128, C], BF16, tag="htok", name="htok")
        # NOTE: keep this on DVE — GpSimd's fused two-op tensor_scalar ucode
        # runs ~21 cyc/elem (17.8us per call, measured).
        nc.vector.tensor_scalar(
            out=ht, in0=xt, scalar1=mv[:, 0:1], scalar2=rstd,
            op0=mybir.AluOpType.subtract, op1=mybir.AluOpType.mult,
        )
        for cp in range(NCK // 2):
            ps = psT.tile([128, 2, 128], BF16, tag="pst", name="pst")
            nc.tensor.transpose(ps[:, 0, :], ht[:, (2 * cp) * 128:(2 * cp + 1) * 128], ident_sb)
            nc.tensor.transpose(ps[:, 1, :], ht[:, (2 * cp + 1) * 128:(2 * cp + 2) * 128], ident_sb)
            # ACT drains the transpose PSUM: ACT sits idle in the LN stretch.
            nc.scalar.copy(dest(cp, i), ps)


def _build_body(tc, io, taps=None):
    nc = tc.nc
    (x_d, wqk_d, wv_d, bqk_d, wo_d, bo_d, wfc_d, wfc8_d, bfc_d, wp_d, wp8_d,
     bp_d, ident_d, maskt_d, out_d) = io

    x_v = x_d.rearrange("(n p) c -> p n c", p=128)
    out_v = out_d.rearrange("(n p) c -> p n c", p=128)

    import contextlib
    est = contextlib.ExitStack()
    with est:
        # tiny consts first (they gate the first PE transposes), then x tiles.
        const = est.enter_context(tc.tile_pool(name="const", bufs=1))
        ident_sb = const.tile([128, 128], BF16, tag="ident", name="ident_sb")
        nc.sync.dma_start(ident_sb, ident_d)
        maskt_sb = const.tile([128, 128], BF16, tag="maskt", name="maskt_sb")
        nc.sync.dma_start(maskt_sb, maskt_d)

        x_pool = est.enter_context(tc.tile_pool(name="xp", bufs=1))
        x_sb = x_pool.tile([128, NT, C], F32, tag="x", name="x_sb")
        for i in range(NT):
            # half-tile DMAs: LN1's first bn_stats only needs columns 0:512,
            # so tile 0's stats chain starts half a tile-DMA earlier.
            nc.sync.dma_start(x_sb[:, i, 0:512], x_v[:, i, 0:512])
            nc.sync.dma_start(x_sb[:, i, 512:1024], x_v[:, i, 512:1024])

        ones_sb = const.tile([1, 128], BF16, tag="ones", name="ones_sb")
        nc.vector.memset(ones_sb, 1.0)
        onesf_sb = const.tile([1, 64], F32, tag="onesf", name="onesf_sb")
        nc.vector.memset(onesf_sb, 1.0)
        eps_sb = const.tile([128, 1], F32, tag="eps", name="eps_sb")
        nc.vector.memset(eps_sb, 1e-5)
        bqk_sb = const.tile([128, 16], F32, tag="bqk", name="bqk_sb")
        nc.sync.dma_start(bqk_sb, bqk_d.rearrange("(n p) -> p n", p=128))
        bfc_sb = const.tile([128, 32], F32, tag="bfc", name="bfc_sb")
        nc.sync.dma_start(bfc_sb, bfc_d.rearrange("(n p) -> p n", p=128))
        bp_sb = const.tile([128, 8], F32, tag="bp", name="bp_sb")
        nc.sync.dma_start(bp_sb, bp_d.rearrange("(n p) -> p n", p=128))
        bo_sb = const.tile([1, C], BF16, tag="bo", name="bo_sb")
        nc.sync.dma_start(bo_sb, bo_d.rearrange("(a n) -> a n", a=1))

        ln_small = est.enter_context(tc.tile_pool(name="lnsmall", bufs=3))
        yT_pool = est.enter_context(tc.tile_pool(name="ytp", bufs=1))
        yT = yT_pool.tile([128, NCK, T], F8, tag="yT", name="yT")
        # FC weight pool sits BELOW the attention pools on the stack
        # allocator so it survives est_attn.close() without pinning the
        # freed attention space; fg0's DMA is issued at attention start.
        wf_pool = est.enter_context(tc.tile_pool(name="wf1", bufs=10))
        # Wp weight pool likewise (cg0 head is prefetched during FC).
        wp_pool = est.enter_context(tc.tile_pool(name="wpp", bufs=12))
        est_attn = est.enter_context(contextlib.ExitStack())
        attn_pool = est_attn.enter_context(tc.tile_pool(name="attnp", bufs=1))
        # k feature-major, two heads packed per 128-row chunk (as produced).
        kT_sb = attn_pool.tile([128, NCK, T], BF16, tag="kT", name="kT_sb")
        # q stored per-head: head h occupies partitions [64*(h%2), +64) of its
        # chunk, the other 64 rows stay ZERO. The scores matmul can then use
        # the full 128-row k chunk as lhsT (junk rows hit zero q rows), keeping
        # the PE at K=128 so the HAM clock gate sees a busy array (K=64
        # matmuls left the whole attention phase throttled to 1.2 GHz).
        qT2 = attn_pool.tile([128, H, T], BF16, tag="qT2", name="qT2")
        # v with one ones-column per head: PV is a single M=65 matmul whose
        # 65th output row is the softmax row-sum.
        v_sb = attn_pool.tile([128, NT, H, D + 1], BF16, tag="v", name="v_sb")
        nc.vector.memset(v_sb[:, :, :, D:D + 1], 1.0)
        # Warm the ACT Sqrt table set while x loads (first LN1 stat otherwise
        # pays the ~2.7us table DMA on the critical x->hT chain).
        warm_sb = const.tile([1, 1], F32, tag="warm", name="warm_sb")
        nc.scalar.activation(warm_sb, eps_sb[0:1, 0:1], AF.Sqrt, bias=0.0, scale=1.0)

        # ---------------- phase 1: load x, LN1, transpose h ----------------
        with tc.tile_pool(name="hTp", bufs=1) as hT_pool, \
             tc.tile_pool(name="psT1", bufs=2, space="PSUM") as psT1, \
             tc.tile_pool(name="psA1", bufs=6, space="PSUM") as psA1, \
             tc.tile_pool(name="wq1", bufs=16) as wq_pool:
            hT = hT_pool.tile([128, NCK, T], F8, tag="hT", name="hT")
            wv_sb = hT_pool.tile([128, NCP, 2, C], F8, tag="wv", name="wv_sb")
            nc.sync.dma_start(wv_sb, wv_d)
            for i in range(NT):
                _ln_tile(nc, (ln_small, ln_small, psT1), x_sb,
                         lambda cp, i: hT[:, 2 * cp:2 * cp + 2, i * 128:(i + 1) * 128],
                         ident_sb, eps_sb, i)

            # ---------------- phase 2: qkv projections (fp8 DoubleRow) ----------------
            # v token-major first (only needs per-token-tile hT, so PE warms up
            # while the qk feature groups' weights stream in). The two nsp
            # halves share each lhsT load.
            for ti in range(NT):
                pv = [psA1.tile([128, 512], F32, tag="psqkv", name="psqkv")
                      for _ in range(2)]
                for cp in range(NCP):
                    lt = hT[:, 2 * cp:2 * cp + 2, ti * 128:(ti + 1) * 128]
                    for nsp in range(2):
                        nc.tensor.matmul(
                            pv[nsp], lhsT=lt,
                            rhs=wv_sb[:, cp, :, nsp * 512:(nsp + 1) * 512],
                            start=(cp == 0), stop=(cp == NCP - 1), perf_mode=DR,
                        )
                for nsp in range(2):
                    nc.vector.tensor_scalar_mul(
                        v_sb[:, ti, nsp * 8:(nsp + 1) * 8, 0:D],
                        pv[nsp].rearrange("p (h d) -> p h d", h=8), 1.0 / WS,
                    )
            # Zero qT2's unused head-halves on GpSimd (it is otherwise idle,
            # and this used to be ~32 ACT ops / a DVE memset that delayed LN1).
            nc.gpsimd.memset(qT2[64:128, 0:H:2, :], 0.0)
            nc.gpsimd.memset(qT2[0:64, 1:H:2, :], 0.0)
            # q,k feature-major: qkT[f, t] = sum_c Wqk[c, f] * hT[c, t]  (+bias via ACT)
            # Feature groups ordered so q-chunk / k-chunk pairs of the low heads
            # land first (heads can start scoring before all of qk is done).
            for fg in (0, 2, 1, 3):  # 512-wide feature groups over 2C
                wts = []
                for cp in range(NCP):
                    wt = wq_pool.tile([128, 2, 512], F8, tag="wqk", name="wqkt")
                    nc.sync.dma_start(wt, wqk_d[cp, :, :, fg * 512:(fg + 1) * 512])
                    wts.append(wt)
                for fl in range(4):
                    fn = fg * 4 + fl
                    pq = [psA1.tile([128, 512], F32, tag="psqkv", name="psqkv")
                          for _ in range(2)]
                    for cp in range(NCP):
                        lt = wts[cp][:, :, fl * 128:(fl + 1) * 128]
                        for tsp in range(2):
                            nc.tensor.matmul(
                                pq[tsp], lhsT=lt,
                                rhs=hT[:, 2 * cp:2 * cp + 2, tsp * 512:(tsp + 1) * 512],
                                start=(cp == 0), stop=(cp == NCP - 1), perf_mode=DR,
                            )
                    for tsp in range(2):
                        ps = pq[tsp]
                        sl = slice(tsp * 512, (tsp + 1) * 512)
                        if fn < NCK:  # q chunk -> per-head halves of qT2
                            nc.scalar.activation(
                                qT2[0:64, 2 * fn, sl], ps[0:64, :],
                                AF.Identity, bias=bqk_sb[0:64, fn:fn + 1], scale=1.0 / WS,
                            )
                            nc.scalar.activation(
                                qT2[64:128, 2 * fn + 1, sl], ps[64:128, :],
                                AF.Identity, bias=bqk_sb[64:128, fn:fn + 1], scale=1.0 / WS,
                            )
                        else:  # k chunk -> DVE (ACT saturates on the q writes)
                            nc.vector.tensor_scalar(
                                out=kT_sb[:, fn - NCK, sl], in0=ps,
                                scalar1=1.0 / WS, scalar2=bqk_sb[:, fn:fn + 1],
                                op0=mybir.AluOpType.mult, op1=mybir.AluOpType.add,
                            )

        # Prefetch Wo into the space wv_sb just released; the 1MB DMA runs
        # behind the attention phase instead of stalling its epilogue.
        wo_pool = est_attn.enter_context(tc.tile_pool(name="wop", bufs=1))
        wo_sb = wo_pool.tile([128, NCP, 2, C], F8, tag="wo", name="wo_sb")
        nc.sync.dma_start(wo_sb, wo_d)
        # Warm the ACT Exp table set now: phase-2 epilogues are Identity
        # (present in every set), so the swap runs behind them instead of
        # stalling the first softmax EXP of the attention phase.
        nc.scalar.activation(warm_sb, eps_sb[0:1, 0:1], AF.Exp, bias=0.0, scale=1.0)

        # FC weights, group 0: issue the DMA now so it streams during
        # attention instead of gating the first FC matmul after LN2.
        wf_fg0 = []
        for ck in range(NCK8):
            wt = wf_pool.tile([128, 512], BF16, tag="wfc", name="wfct")
            nc.sync.dma_start(wt, wfc_d[ck * 128:(ck + 1) * 128, 0:512])
            wf_fg0.append(wt)

        # ---------------- phase 3: attention (per head) ----------------
        with tc.tile_pool(name="ptp", bufs=2) as pt_pool, \
             tc.tile_pool(name="asml", bufs=2) as asml, \
             tc.tile_pool(name="psS", bufs=2, space="PSUM") as psS, \
             tc.tile_pool(name="psY", bufs=2, space="PSUM") as psY:
            inv_sqrt_c = 1.0 / math.sqrt(C)

            def scores_phase(h):
                hc = h // 2
                qT = qT2[:, h, :]               # zero-padded to 128 rows
                kT = kT_sb[:, hc, :]            # full chunk; junk rows hit q zeros
                PT = pt_pool.tile([128, NT, T], BF16, tag="pt", name="PT")
                for j in range(NT):
                    lo = j * 128
                    ss = psS.tile([128, T], F32, tag="st", name="ss")
                    for (a, b) in _pieces(lo, T):
                        nc.tensor.matmul(
                            ss[:, a:b], lhsT=kT[:, lo:lo + 128], rhs=qT[:, a:b],
                            start=True, stop=(a != lo), skip_group_check=True,
                        )
                    # causal mask: accumulate -1280 on the diagonal block's
                    # sub-diagonal (PE, vs identity) so EXP underflows it —
                    # replaces a per-(h,j) DVE/GpSimd mask multiply on PT.
                    nc.tensor.matmul(
                        ss[:, lo:lo + 128], lhsT=maskt_sb, rhs=ident_sb,
                        start=False, stop=True, skip_group_check=True,
                    )
                    nc.scalar.activation(PT[:, j, lo:T], ss[:, lo:T], AF.Exp, scale=inv_sqrt_c)
                return PT

            def pv_phase(h, PT):
                yps = psY.tile([65, T], F32, tag="y", name="yps")
                for j in range(NT):
                    lv = v_sb[:, j, h, :]
                    for (a, b) in _pieces(j * 128, T):
                        last = (j == min(NT - 1, (b - 1) // 128))
                        nc.tensor.matmul(
                            yps[:, a:b], lhsT=lv, rhs=PT[:, j, a:b],
                            start=(j == 0), stop=last, skip_group_check=True,
                        )
                # Row-sum (the ones-column output) to a base-0 tile for the
                # custom-DVE reciprocal; unnormalized y stays in PSUM and is
                # read directly by the normalize multiply.
                rsum = asml.tile([1, T], F32, tag="rsum", bufs=2, name="rsum")
                nc.vector.tensor_copy(rsum, yps[64:65, :])
                return yps, rsum

            def epi_phase(h, yps, rsum):
                po = 64 * (h % 2)
                hc = h // 2
                rb1 = asml.tile([1, T], F32, tag="rb1", bufs=2, name="rb1")
                nc.vector.reciprocal_approx_fast(rb1, rsum)
                # broadcast the reciprocal row to 64 partitions on GpSimd
                # (idle), freeing both PE (old ones-matmul bcast) and DVE.
                rbs = asml.tile([64, T], F32, tag="rbs", name="rbs")
                nc.gpsimd.partition_broadcast(rbs, rb1)
                if taps is not None:
                    nc.sync.dma_start(taps["sums"][h:h + 1, :], rsum[0:1, :])
                    nc.sync.dma_start(taps["recips"][h:h + 1, :], rb1[0:1, :])
                    nc.sync.dma_start(taps["rbsrow"][h:h + 1, :], rbs[0:1, :])
                nc.vector.tensor_mul(yT[po:po + 64, hc, :], yps[0:64, :], rbs)

            # 3-stage pipeline: scores(h) | PV(h-1) | epilogue(h-2). The PE
            # never waits on the reciprocal chain: by the time the tiny
            # broadcast matmuls of head h-2 reach the in-order PE queue their
            # inputs have long been ready.
            pts = {}
            pvres = {}
            for h in range(H):
                pts[h] = scores_phase(h)
                if h - 1 >= 0:
                    pvres[h - 1] = pv_phase(h - 1, pts.pop(h - 1))
                if h - 2 >= 0:
                    epi_phase(h - 2, *pvres.pop(h - 2))
            epi_phase(H - 2, *pvres.pop(H - 2))  # before PV(15): fills a PE bubble
            pvres[H - 1] = pv_phase(H - 1, pts.pop(H - 1))
            epi_phase(H - 1, *pvres.pop(H - 1))

        x2 = x_sb  # attention residual is written in place

        if taps is not None:
            nc.sync.dma_start(taps["yT"], yT)

        # ---------------- phase 4: attention out-proj + residual ----------------
        with tc.tile_pool(name="psA2", bufs=4, space="PSUM") as psA2:
            for ti in range(NT):
                po = [psA2.tile([128, 512], F32, tag="pswo", name="pswo")
                      for _ in range(2)]
                for cp in range(NCP):
                    lt = yT[:, 2 * cp:2 * cp + 2, ti * 128:(ti + 1) * 128]
                    for nsp in range(2):
                        nc.tensor.matmul(
                            po[nsp], lhsT=lt,
                            rhs=wo_sb[:, cp, :, nsp * 512:(nsp + 1) * 512],
                            start=(cp == 0), stop=False, perf_mode=DR,
                        )
                for nsp in range(2):
                    nc.tensor.matmul(po[nsp], lhsT=ones_sb[0:1, 0:128],
                                     rhs=bo_sb[0:1, nsp * 512:(nsp + 1) * 512],
                                     start=False, stop=True)
                    nc.vector.tensor_add(
                        x2[:, ti, nsp * 512:(nsp + 1) * 512], po[nsp],
                        x_sb[:, ti, nsp * 512:(nsp + 1) * 512],
                    )

        est_attn.close()  # free kT/qT2/v/wo space before MLP tensors
        # ---------------- phase 5/6: LN2 + FC(gelu) ----------------
        # The MLP runs mixed precision: the last C/4 of the FC contraction and
        # the last 2C of the Wp contraction are fp8 DoubleRow (more would
        # breach the 2e-2 rel-err gate). All MLP weights are pre-scaled by 32
        # (exact power of two) so the fp8 rows sit in e4m3 normal range; the
        # PSUM epilogues compensate with scale=1/32.
        mlp_pool = est.enter_context(tc.tile_pool(name="mlpp", bufs=1))
        mT = mlp_pool.tile([128, KN8, T], BF16, tag="mT", name="mT")
        mT8 = mlp_pool.tile([128, 4 * NCK - KN8, T], F8, tag="mT8", name="mT8")
        outT = mlp_pool.tile([128, NCK, T], BF16, tag="outT", name="outT")
        with tc.tile_pool(name="h2Tp", bufs=1) as h2T_pool, \
             tc.tile_pool(name="psT2", bufs=2, space="PSUM") as psT2, \
             tc.tile_pool(name="psA3", bufs=4, space="PSUM") as psA3:
            h2T = h2T_pool.tile([128, NCK8, T], BF16, tag="h2T", name="h2T")
            h2T8 = h2T_pool.tile([128, 2, T], F8, tag="h2T8", name="h2T8")

            def h2_dest(cp, i):
                tsl = slice(i * 128, (i + 1) * 128)
                if 2 * cp < NCK8:
                    return h2T[:, 2 * cp:2 * cp + 2, tsl]
                return h2T8[:, 0:2, tsl]

            for i in range(NT):
                _ln_tile(nc, (ln_small, ln_small, psT2), x2, h2_dest,
                         ident_sb, eps_sb, i)
            # Wp output-group 0 weights: prefetch during FC so the Wp phase
            # doesn't start with a weight-DMA stall.
            wp_pre = []
            for kn in range(8):
                wt = wp_pool.tile([128, 256], BF16, tag="wp", name="wpt")
                nc.sync.dma_start(wt, wp_d[kn * 128:(kn + 1) * 128, 0:256])
                wp_pre.append(wt)
            for fg in range(8):  # 512-wide feature groups over 4C
                if fg == 0:
                    wts = wf_fg0
                else:
                    wts = []
                    for ck in range(NCK8):
                        wt = wf_pool.tile([128, 512], BF16, tag="wfc", name="wfct")
                        nc.sync.dma_start(wt, wfc_d[ck * 128:(ck + 1) * 128, fg * 512:(fg + 1) * 512])
                        wts.append(wt)
                wt8 = wf_pool.tile([128, 2, 512], F8, tag="wfc8", bufs=3, name="wfc8t")
                nc.sync.dma_start(wt8, wfc8_d[:, 0, :, fg * 512:(fg + 1) * 512])
                for fl in range(4):
                    fn = fg * 4 + fl
                    pf = [psA3.tile([128, 512], F32, tag="psfc", name="psfc")
                          for _ in range(2)]
                    for ck in range(NCK8):
                        lt = wts[ck][:, fl * 128:(fl + 1) * 128]
                        for tsp in range(2):
                            nc.tensor.matmul(
                                pf[tsp], lhsT=lt,
                                rhs=h2T[:, ck, tsp * 512:(tsp + 1) * 512],
                                start=(ck == 0), stop=False,
                            )
                    lt8 = wt8[:, :, fl * 128:(fl + 1) * 128]
                    for tsp in range(2):
                        nc.tensor.matmul(
                            pf[tsp], lhsT=lt8,
                            rhs=h2T8[:, 0:2, tsp * 512:(tsp + 1) * 512],
                            start=False, stop=True, perf_mode=DR,
                        )
                    for tsp in range(2):
                        dst, dn = (mT, fn) if fn < KN8 else (mT8, fn - KN8)
                        nc.scalar.activation(
                            dst[:, dn, tsp * 512:(tsp + 1) * 512], pf[tsp],
                            AF.Gelu_apprx_tanh, bias=bfc_sb[:, fn:fn + 1], scale=1.0 / WS,
                        )

        # ---------------- phase 7: Wp (feature-major out) ----------------
        with tc.tile_pool(name="psW", bufs=4, space="PSUM") as psW, \
             tc.tile_pool(name="psT3", bufs=2, space="PSUM") as psT3, \
             tc.tile_pool(name="outp", bufs=8) as out_pool:

            def out_quarter(cg, outts):
                # transpose-back + residual + store for output column quarter
                # `cg` (the two outT chunks just produced); emitted mid-Wp so
                # the PE transposes run while the array is dense and warm, and
                # each 1MB output DMA overlaps the remaining Wp compute.
                for ti in range(NT):
                    if cg == 0:
                        outts.append(out_pool.tile([128, C], F32, tag="osb", name="outt"))
                    outt = outts[ti]
                    ps2 = psT3.tile([128, 2, 128], BF16, tag="pst3", name="ps2")
                    for cl in range(2):
                        cj = cg * 2 + cl
                        nc.tensor.transpose(ps2[:, cl, :], outT[:, cj, ti * 128:(ti + 1) * 128], ident_sb)
                    nc.vector.tensor_add(
                        outt[:, cg * 256:(cg + 1) * 256].rearrange("p (a b) -> p a b", a=2),
                        ps2,
                        x2[:, ti, cg * 256:(cg + 1) * 256].rearrange("p (a b) -> p a b", a=2),
                    )
                    nc.sync.dma_start(out_v[:, ti, cg * 256:(cg + 1) * 256],
                                      outt[:, cg * 256:(cg + 1) * 256])

            outts = []
            for cg in range(4):  # output feature groups of 256
                pss = [[psW.tile([128, 512], F32, tag="pswp", name="pswp")
                        for _ in range(2)] for _ in range(2)]
                for kn in range(KN8):  # bf16 contraction chunks
                    if cg == 0 and kn < 8:
                        wt = wp_pre[kn]
                    else:
                        wt = wp_pool.tile([128, 256], BF16, tag="wp", name="wpt")
                        nc.scalar.dma_start(wt, wp_d[kn * 128:(kn + 1) * 128, cg * 256:(cg + 1) * 256])
                    for cl in range(2):
                        for tsp in range(2):
                            nc.tensor.matmul(
                                pss[cl][tsp], lhsT=wt[:, cl * 128:(cl + 1) * 128],
                                rhs=mT[:, kn, tsp * 512:(tsp + 1) * 512],
                                start=(kn == 0), stop=False,
                            )
                for kp in range((4 * NCK - KN8) // 2):  # fp8 DoubleRow pair chunks
                    wt8 = wp_pool.tile([128, 2, 256], F8, tag="wp8", bufs=6, name="wp8t")
                    nc.scalar.dma_start(wt8, wp8_d[kp, :, :, cg * 256:(cg + 1) * 256])
                    for cl in range(2):
                        for tsp in range(2):
                            nc.tensor.matmul(
                                pss[cl][tsp], lhsT=wt8[:, :, cl * 128:(cl + 1) * 128],
                                rhs=mT8[:, 2 * kp:2 * kp + 2, tsp * 512:(tsp + 1) * 512],
                                start=False, stop=(kp == (4 * NCK - KN8) // 2 - 1), perf_mode=DR,
                            )
                for cl in range(2):
                    cn = cg * 2 + cl
                    for tsp in range(2):
                        nc.scalar.activation(
                            outT[:, cn, tsp * 512:(tsp + 1) * 512], pss[cl][tsp],
                            AF.Identity, bias=bp_sb[:, cn:cn + 1], scale=1.0 / WS,
                        )
                out_quarter(cg, outts)


def build_module(debug_taps=False):
    nc = bacc.Bacc("TRN2", target_bir_lowering=False, debug=False)

    def din(name, shape, dtype):
        return nc.dram_tensor(name, list(shape), dtype, kind="ExternalInput").ap()

    taps = None
    if debug_taps:
        taps = {
            "sums": nc.dram_tensor("dbg_sums", [H, T], F32, kind="ExternalOutput").ap(),
            "recips": nc.dram_tensor("dbg_recips", [H, T], F32, kind="ExternalOutput").ap(),
            "rbsrow": nc.dram_tensor("dbg_rbsrow", [H, T], F32, kind="ExternalOutput").ap(),
            "yT": nc.dram_tensor("dbg_yT", [128, NCK, T], F8, kind="ExternalOutput").ap(),
        }

    io = (
        din("x", (T, C), F32),
        din("wqk", (NCP, 128, 2, 2 * C), F8),
        din("wv", (128, NCP, 2, C), F8),
        din("bqk", (2 * C,), F32),
        din("wo", (128, NCP, 2, C), F8),
        din("bo", (C,), BF16),
        din("wfc", (C - 256, 4 * C), BF16),
        din("wfc8", (128, 1, 2, 4 * C), F8),
        din("bfc", (4 * C,), F32),
        din("wp", (C, C), BF16),
        din("wp8", (3 * NCP, 128, 2, C), F8),
        din("bp", (C,), F32),
        din("ident", (128, 128), BF16),
        din("maskt", (128, 128), BF16),
        nc.dram_tensor("out", [T, C], F32, kind="ExternalOutput").ap(),
    )
    with tile.TileContext(nc) as tc:
        _build_body(tc, io, taps=taps)
    nc.compile()
    return nc


def host_prepare(inputs):
    """Fold LN affine params / v-bias into weights; cast matmul weights to
    bf16 (MLP) / pair-interleaved fp8e4 (attention projections)."""
    bf = ml_dtypes.bfloat16
    f8 = ml_dtypes.float8_e4m3
    x = np.asarray(inputs["x"], np.float32)
    Wqkv = np.asarray(inputs["Wqkv"], np.float64)
    bqkv = np.asarray(inputs["bqkv"], np.float64)
    Wo = np.asarray(inputs["Wo"], np.float64)
    bo = np.asarray(inputs["bo"], np.float64)
    ln1_w = np.asarray(inputs["ln1_w"], np.float64)
    ln1_b = np.asarray(inputs["ln1_b"], np.float64)
    ln2_w = np.asarray(inputs["ln2_w"], np.float64)
    ln2_b = np.asarray(inputs["ln2_b"], np.float64)
    Wfc = np.asarray(inputs["Wfc"], np.float64)
    bfc = np.asarray(inputs["bfc"], np.float64)
    Wp = np.asarray(inputs["Wp"], np.float64)
    bp = np.asarray(inputs["bp"], np.float64)

    Wqkv_f = ln1_w[:, None] * Wqkv
    bqkv_f = bqkv + ln1_b @ Wqkv
    bo_f = bo + bqkv_f[2 * C:] @ Wo
    Wfc_f = ln2_w[:, None] * Wfc
    bfc_f = bfc + ln2_b @ Wfc

    def pair_cf(w):  # [K, F] -> [K/256, 128, 2, F]  (DoubleRow lhsT layout)
        return np.ascontiguousarray(
            w.reshape(-1, 2, 128, w.shape[1]).transpose(0, 2, 1, 3))

    def pair_pf(w):  # [K, F] -> [128, K/256, 2, F]  (DoubleRow rhs layout)
        return np.ascontiguousarray(
            w.reshape(-1, 2, 128, w.shape[1]).transpose(2, 0, 1, 3))

    common = {
        "wqk": pair_cf(Wqkv_f[:, :2 * C] * WS).astype(f8),
        "wv": pair_pf(Wqkv_f[:, 2 * C:] * WS).astype(f8),
        "bqk": bqkv_f[:2 * C].astype(np.float32),
        "wo": pair_pf(Wo).astype(f8),
        "bo": bo_f.astype(bf),
        # MLP weights are scaled by 32 (exact power of 2; epilogues divide it
        # back out) so their fp8 tail rows sit in e4m3 normal range.
        "wfc": (Wfc_f[:C - 256] * WS).astype(bf),
        "wfc8": pair_pf(Wfc_f[C - 256:] * WS).astype(f8),
        "bfc": bfc_f.astype(np.float32),
        "wp": (Wp[:C] * WS).astype(bf),
        "wp8": pair_cf(Wp[C:] * WS).astype(f8),
        "bp": bp.astype(np.float32),
        "ident": np.eye(128, dtype=bf),
        # additive causal mask, applied as lhsT in a PE accumulate vs ident:
        # ss[k, q] += maskt.T[k, q] = -1280 where k > q; exp(-1280/32) -> ~0
        "maskt": (np.triu(np.ones((128, 128)), k=1) * -1280.0).astype(bf),
    }
    return x, common


_NC_CACHE = None


def get_module():
    global _NC_CACHE
    if _NC_CACHE is None:
        _NC_CACHE = build_module()
    return _NC_CACHE


def run_with_results(inputs, **run_kwargs):
    x, common = host_prepare(inputs)
    nc = get_module()
    in_maps = [dict(common, x=np.ascontiguousarray(x[b])) for b in range(B)]
    res = run_bass_kernel_spmd(nc, in_maps, core_ids=list(range(N_CORES)), **run_kwargs)
    out = np.stack([res.results[b]["out"] for b in range(B)]).astype(np.float32)
    return out, res


def kernel(**inputs):
    return run_with_results(inputs)[0]


# revision 59
# speedup vs baseline: 1.0169x; 1.0169x over previous
"""Transformer block (LN -> causal MHA -> LN -> MLP, residuals) on 8 trn2 NeuronCores.

Data-parallel over batch: each core runs one [T, C] sequence independently
(no collectives). The attention-branch projections (qk, v, Wo) run as
fp8e4 DoubleRow matmuls (2 MACs/cell/cycle, contraction 256 per pass);
layernorm, softmax, residuals and the MLP stay fp32/bf16 — fp8 on the MLP
would push rel-err past the 2e-2 gate, fp8 on the attention branch costs
~1e-3 because softmax output is a near-uniform average (y is ~3% the scale
of the residual stream).

Host-side preprocessing folds the layernorm affine params into the adjacent
matmul weights, folds the V bias through Wo, pre-interleaves the fp8 weight
pairs ([K,2,*] DoubleRow layout), and scales Wqkv by 32 (compensated in the
PSUM epilogues) to keep fp8 weights in e4m3 normal range.
"""

import math
import sys

for _p in ("/opt/trn_rl_repo", "/root/.axon_site/_ro/trn_rl_repo"):
    if _p not in sys.path:
        sys.path.append(_p)

import numpy as np
import ml_dtypes

import concourse.bass as bass
import concourse.mybir as mybir
import concourse.tile as tile
from concourse import bacc
from concourse.bass_utils import run_bass_kernel_spmd

B, T, C, H = 8, 1024, 1024, 16
D = C // H
NT = T // 128          # token tiles
NCK = C // 128         # contraction chunks over C
NCP = NCK // 2         # fp8 DoubleRow chunk-pairs over C
NCK8 = NCK - 2         # bf16 FC-contraction chunks (last pair is fp8)
KN8 = NCK              # bf16 Wp-contraction chunks (last 3/4 is fp8)
F32 = mybir.dt.float32
BF16 = mybir.dt.bfloat16
F8 = mybir.dt.float8e4
DR = mybir.MatmulPerfMode.DoubleRow
AF = mybir.ActivationFunctionType
N_CORES = 8
WS = 32.0              # fp8 weight scale for qk/v (undone in epilogues)


def _pieces(lo, hi, bound=512):
    """Split [lo, hi) at multiples of `bound` (PSUM bank boundaries)."""
    out = []
    a = lo
    while a < hi:
        b = min(hi, (a // bound + 1) * bound)
        out.append((a, b))
        a = b
    return out


def _ln_stats(nc, stat_pool, src, eps_sb, i, bufs=9):
    """Token-tile LN statistics: returns (mean, rstd) tiles for tile i."""
    xt = src[:, i, :]
    stats = stat_pool.tile([128, 2, 6], F32, tag="lnstats", name="lnstats")
    nc.vector.bn_stats(stats[:, 0, :], xt[:, 0:512])
    nc.vector.bn_stats(stats[:, 1, :], xt[:, 512:1024])
    mv = stat_pool.tile([128, 2], F32, tag="lnmv", bufs=bufs, name="lnmv")
    nc.vector.bn_aggr(mv, stats)
    std = stat_pool.tile([128, 1], F32, tag="lnstd", name="lnstd")
    nc.scalar.activation(std, mv[:, 1:2], AF.Sqrt, bias=eps_sb, scale=1.0)
    rstd = stat_pool.tile([128, 1], F32, tag="lnrstd", bufs=bufs, name="lnrstd")
    nc.vector.reciprocal(rstd, std)
    return mv, rstd


def _ln_finish(nc, pools, src, dest, ident_sb, i, mv, rstd):
    """Normalize + feature-major transpose of token-tile i to dest(cp, i)
    (the ACT drain converts to the destination dtype, bf16 or fp8).
    Transposes run in bf16 (hw rejects plain fp8 PE-transposes)."""
    htok_pool, psT = pools
    xt = src[:, i, :]
    ht = htok_pool.tile([128, C], BF16, tag="htok", name="htok")
    # NOTE: keep this on DVE — GpSimd's fused two-op tensor_scalar ucode
    # runs ~21 cyc/elem (17.8us per call, measured).
    nc.vector.tensor_scalar(
        out=ht, in0=xt, scalar1=mv[:, 0:1], scalar2=rstd,
        op0=mybir.AluOpType.subtract, op1=mybir.AluOpType.mult,
    )
    for cp in range(NCK // 2):
        ps = psT.tile([128, 2, 128], BF16, tag="pst", name="pst")
        nc.tensor.transpose(ps[:, 0, :], ht[:, (2 * cp) * 128:(2 * cp + 1) * 128], ident_sb)
        nc.tensor.transpose(ps[:, 1, :], ht[:, (2 * cp + 1) * 128:(2 * cp + 2) * 128], ident_sb)
        # ACT drains the transpose PSUM: ACT sits idle in the LN stretch.
        nc.scalar.copy(dest(cp, i), ps)


def _ln_tile(nc, pools, src, dest, ident_sb, eps_sb, i):
    stat_pool, htok_pool, psT = pools
    mv, rstd = _ln_stats(nc, stat_pool, src, eps_sb, i)
    _ln_finish(nc, (htok_pool, psT), src, dest, ident_sb, i, mv, rstd)


def _build_body(tc, io, taps=None):
    nc = tc.nc
    (x_d, wqk_d, wv_d, bqk_d, wo_d, bo_d, wfc_d, wfc8_d, bfc_d, wp_d, wp8_d,
     bp_d, ident_d, maskt_d, out_d) = io

    x_v = x_d.rearrange("(n p) c -> p n c", p=128)
    out_v = out_d.rearrange("(n p) c -> p n c", p=128)

    import contextlib
    est = contextlib.ExitStack()
    with est:
        # tiny consts first (they gate the first PE transposes), then x tiles.
        const = est.enter_context(tc.tile_pool(name="const", bufs=1))
        ident_sb = const.tile([128, 128], BF16, tag="ident", name="ident_sb")
        nc.sync.dma_start(ident_sb, ident_d)
        maskt_sb = const.tile([128, 128], BF16, tag="maskt", name="maskt_sb")
        nc.sync.dma_start(maskt_sb, maskt_d)

        x_pool = est.enter_context(tc.tile_pool(name="xp", bufs=1))
        x_sb = x_pool.tile([128, NT, C], F32, tag="x", name="x_sb")
        for i in range(NT):
            # half-tile DMAs: LN1's first bn_stats only needs columns 0:512,
            # so tile 0's stats chain starts half a tile-DMA earlier.
            nc.sync.dma_start(x_sb[:, i, 0:512], x_v[:, i, 0:512])
            nc.sync.dma_start(x_sb[:, i, 512:1024], x_v[:, i, 512:1024])

        ones_sb = const.tile([1, 128], BF16, tag="ones", name="ones_sb")
        nc.vector.memset(ones_sb, 1.0)
        onesf_sb = const.tile([1, 64], F32, tag="onesf", name="onesf_sb")
        nc.vector.memset(onesf_sb, 1.0)
        eps_sb = const.tile([128, 1], F32, tag="eps", name="eps_sb")
        nc.vector.memset(eps_sb, 1e-5)
        bqk_sb = const.tile([128, 16], F32, tag="bqk", name="bqk_sb")
        nc.sync.dma_start(bqk_sb, bqk_d.rearrange("(n p) -> p n", p=128))
        bfc_sb = const.tile([128, 32], F32, tag="bfc", name="bfc_sb")
        nc.sync.dma_start(bfc_sb, bfc_d.rearrange("(n p) -> p n", p=128))
        bp_sb = const.tile([128, 8], F32, tag="bp", name="bp_sb")
        nc.sync.dma_start(bp_sb, bp_d.rearrange("(n p) -> p n", p=128))
        bo_sb = const.tile([1, C], BF16, tag="bo", name="bo_sb")
        nc.sync.dma_start(bo_sb, bo_d.rearrange("(a n) -> a n", a=1))

        ln_small = est.enter_context(tc.tile_pool(name="lnsmall", bufs=3))
        yT_pool = est.enter_context(tc.tile_pool(name="ytp", bufs=1))
        yT = yT_pool.tile([128, NCK, T], F8, tag="yT", name="yT")
        # FC weight pool sits BELOW the attention pools on the stack
        # allocator so it survives est_attn.close() without pinning the
        # freed attention space; fg0's DMA is issued at attention start.
        wf_pool = est.enter_context(tc.tile_pool(name="wf1", bufs=10))
        # Wp weight pool likewise (cg0 head is prefetched during FC).
        wp_pool = est.enter_context(tc.tile_pool(name="wpp", bufs=12))
        est_attn = est.enter_context(contextlib.ExitStack())
        attn_pool = est_attn.enter_context(tc.tile_pool(name="attnp", bufs=1))
        # k feature-major, two heads packed per 128-row chunk (as produced).
        kT_sb = attn_pool.tile([128, NCK, T], BF16, tag="kT", name="kT_sb")
        # q stored per-head: head h occupies partitions [64*(h%2), +64) of its
        # chunk, the other 64 rows stay ZERO. The scores matmul can then use
        # the full 128-row k chunk as lhsT (junk rows hit zero q rows), keeping
        # the PE at K=128 so the HAM clock gate sees a busy array (K=64
        # matmuls left the whole attention phase throttled to 1.2 GHz).
        qT2 = attn_pool.tile([128, H, T], BF16, tag="qT2", name="qT2")
        # v with one ones-column per head: PV is a single M=65 matmul whose
        # 65th output row is the softmax row-sum.
        v_sb = attn_pool.tile([128, NT, H, D + 1], BF16, tag="v", name="v_sb")
        nc.vector.memset(v_sb[:, :, :, D:D + 1], 1.0)
        # Warm the ACT Sqrt table set while x loads (first LN1 stat otherwise
        # pays the ~2.7us table DMA on the critical x->hT chain).
        warm_sb = const.tile([1, 1], F32, tag="warm", name="warm_sb")
        nc.scalar.activation(warm_sb, eps_sb[0:1, 0:1], AF.Sqrt, bias=0.0, scale=1.0)

        # ---------------- phase 1: load x, LN1, transpose h ----------------
        with tc.tile_pool(name="hTp", bufs=1) as hT_pool, \
             tc.tile_pool(name="psT1", bufs=2, space="PSUM") as psT1, \
             tc.tile_pool(name="psA1", bufs=6, space="PSUM") as psA1, \
             tc.tile_pool(name="wq1", bufs=16) as wq_pool:
            hT = hT_pool.tile([128, NCK, T], F8, tag="hT", name="hT")
            wv_sb = hT_pool.tile([128, NCP, 2, C], F8, tag="wv", name="wv_sb")
            nc.sync.dma_start(wv_sb, wv_d)
            for i in range(NT):
                _ln_tile(nc, (ln_small, ln_small, psT1), x_sb,
                         lambda cp, i: hT[:, 2 * cp:2 * cp + 2, i * 128:(i + 1) * 128],
                         ident_sb, eps_sb, i)

            # ---------------- phase 2: qkv projections (fp8 DoubleRow) ----------------
            # v token-major first (only needs per-token-tile hT, so PE warms up
            # while the qk feature groups' weights stream in). The two nsp
            # halves share each lhsT load.
            for ti in range(NT):
                pv = [psA1.tile([128, 512], F32, tag="psqkv", name="psqkv")
                      for _ in range(2)]
                for cp in range(NCP):
                    lt = hT[:, 2 * cp:2 * cp + 2, ti * 128:(ti + 1) * 128]
                    for nsp in range(2):
                        nc.tensor.matmul(
                            pv[nsp], lhsT=lt,
                            rhs=wv_sb[:, cp, :, nsp * 512:(nsp + 1) * 512],
                            start=(cp == 0), stop=(cp == NCP - 1), perf_mode=DR,
                        )
                for nsp in range(2):
                    nc.vector.tensor_scalar_mul(
                        v_sb[:, ti, nsp * 8:(nsp + 1) * 8, 0:D],
                        pv[nsp].rearrange("p (h d) -> p h d", h=8), 1.0 / WS,
                    )
            # Zero qT2's unused head-halves on GpSimd (it is otherwise idle,
            # and this used to be ~32 ACT ops / a DVE memset that delayed LN1).
            nc.gpsimd.memset(qT2[64:128, 0:H:2, :], 0.0)
            nc.gpsimd.memset(qT2[0:64, 1:H:2, :], 0.0)
            # q,k feature-major: qkT[f, t] = sum_c Wqk[c, f] * hT[c, t]  (+bias via ACT)
            # Feature groups ordered so q-chunk / k-chunk pairs of the low heads
            # land first (heads can start scoring before all of qk is done).
            for fg in (0, 2, 1, 3):  # 512-wide feature groups over 2C
                wts = []
                for cp in range(NCP):
                    wt = wq_pool.tile([128, 2, 512], F8, tag="wqk", name="wqkt")
                    nc.sync.dma_start(wt, wqk_d[cp, :, :, fg * 512:(fg + 1) * 512])
                    wts.append(wt)
                for fl in range(4):
                    fn = fg * 4 + fl
                    pq = [psA1.tile([128, 512], F32, tag="psqkv", name="psqkv")
                          for _ in range(2)]
                    for cp in range(NCP):
                        lt = wts[cp][:, :, fl * 128:(fl + 1) * 128]
                        for tsp in range(2):
                            nc.tensor.matmul(
                                pq[tsp], lhsT=lt,
                                rhs=hT[:, 2 * cp:2 * cp + 2, tsp * 512:(tsp + 1) * 512],
                                start=(cp == 0), stop=(cp == NCP - 1), perf_mode=DR,
                            )
                    for tsp in range(2):
                        ps = pq[tsp]
                        sl = slice(tsp * 512, (tsp + 1) * 512)
                        if fn < NCK:  # q chunk -> per-head halves of qT2
                            nc.scalar.activation(
                                qT2[0:64, 2 * fn, sl], ps[0:64, :],
                                AF.Identity, bias=bqk_sb[0:64, fn:fn + 1], scale=1.0 / WS,
                            )
                            nc.scalar.activation(
                                qT2[64:128, 2 * fn + 1, sl], ps[64:128, :],
                                AF.Identity, bias=bqk_sb[64:128, fn:fn + 1], scale=1.0 / WS,
                            )
                        else:  # k chunk -> DVE (ACT saturates on the q writes)
                            nc.vector.tensor_scalar(
                                out=kT_sb[:, fn - NCK, sl], in0=ps,
                                scalar1=1.0 / WS, scalar2=bqk_sb[:, fn:fn + 1],
                                op0=mybir.AluOpType.mult, op1=mybir.AluOpType.add,
                            )

        # Prefetch Wo into the space wv_sb just released; the 1MB DMA runs
        # behind the attention phase instead of stalling its epilogue.
        wo_pool = est_attn.enter_context(tc.tile_pool(name="wop", bufs=1))
        wo_sb = wo_pool.tile([128, NCP, 2, C], F8, tag="wo", name="wo_sb")
        nc.sync.dma_start(wo_sb, wo_d)
        # Warm the ACT Exp table set now: phase-2 epilogues are Identity
        # (present in every set), so the swap runs behind them instead of
        # stalling the first softmax EXP of the attention phase.
        nc.scalar.activation(warm_sb, eps_sb[0:1, 0:1], AF.Exp, bias=0.0, scale=1.0)

        # FC weights, group 0: issue the DMA now so it streams during
        # attention instead of gating the first FC matmul after LN2.
        wf_fg0 = []
        for ck in range(NCK8):
            wt = wf_pool.tile([128, 512], BF16, tag="wfc", name="wfct")
            nc.sync.dma_start(wt, wfc_d[ck * 128:(ck + 1) * 128, 0:512])
            wf_fg0.append(wt)

        # ---------------- phase 3: attention (per head) ----------------
        with tc.tile_pool(name="ptp", bufs=2) as pt_pool, \
             tc.tile_pool(name="asml", bufs=2) as asml, \
             tc.tile_pool(name="psS", bufs=2, space="PSUM") as psS, \
             tc.tile_pool(name="psY", bufs=2, space="PSUM") as psY:
            inv_sqrt_c = 1.0 / math.sqrt(C)

            def scores_phase(h):
                hc = h // 2
                qT = qT2[:, h, :]               # zero-padded to 128 rows
                kT = kT_sb[:, hc, :]            # full chunk; junk rows hit q zeros
                PT = pt_pool.tile([128, NT, T], BF16, tag="pt", name="PT")
                for j in range(NT):
                    lo = j * 128
                    ss = psS.tile([128, T], F32, tag="st", name="ss")
                    for (a, b) in _pieces(lo, T):
                        nc.tensor.matmul(
                            ss[:, a:b], lhsT=kT[:, lo:lo + 128], rhs=qT[:, a:b],
                            start=True, stop=(a != lo), skip_group_check=True,
                        )
                    # causal mask: accumulate -1280 on the diagonal block's
                    # sub-diagonal (PE, vs identity) so EXP underflows it —
                    # replaces a per-(h,j) DVE/GpSimd mask multiply on PT.
                    nc.tensor.matmul(
                        ss[:, lo:lo + 128], lhsT=maskt_sb, rhs=ident_sb,
                        start=False, stop=True, skip_group_check=True,
                    )
                    nc.scalar.activation(PT[:, j, lo:T], ss[:, lo:T], AF.Exp, scale=inv_sqrt_c)
                return PT

            def pv_phase(h, PT):
                yps = psY.tile([65, T], F32, tag="y", name="yps")
                for j in range(NT):
                    lv = v_sb[:, j, h, :]
                    for (a, b) in _pieces(j * 128, T):
                        last = (j == min(NT - 1, (b - 1) // 128))
                        nc.tensor.matmul(
                            yps[:, a:b], lhsT=lv, rhs=PT[:, j, a:b],
                            start=(j == 0), stop=last, skip_group_check=True,
                        )
                # Row-sum (the ones-column output) to a base-0 tile for the
                # custom-DVE reciprocal; unnormalized y stays in PSUM and is
                # read directly by the normalize multiply.
                rsum = asml.tile([1, T], F32, tag="rsum", bufs=2, name="rsum")
                nc.vector.tensor_copy(rsum, yps[64:65, :])
                return yps, rsum

            def epi_phase(h, yps, rsum):
                po = 64 * (h % 2)
                hc = h // 2
                rb1 = asml.tile([1, T], F32, tag="rb1", bufs=2, name="rb1")
                nc.vector.reciprocal_approx_fast(rb1, rsum)
                # broadcast the reciprocal row to 64 partitions on GpSimd
                # (idle), freeing both PE (old ones-matmul bcast) and DVE.
                rbs = asml.tile([64, T], F32, tag="rbs", name="rbs")
                nc.gpsimd.partition_broadcast(rbs, rb1)
                if taps is not None:
                    nc.sync.dma_start(taps["sums"][h:h + 1, :], rsum[0:1, :])
                    nc.sync.dma_start(taps["recips"][h:h + 1, :], rb1[0:1, :])
                    nc.sync.dma_start(taps["rbsrow"][h:h + 1, :], rbs[0:1, :])
                nc.vector.tensor_mul(yT[po:po + 64, hc, :], yps[0:64, :], rbs)

            # 3-stage pipeline: scores(h) | PV(h-1) | epilogue(h-2). The PE
            # never waits on the reciprocal chain: by the time the tiny
            # broadcast matmuls of head h-2 reach the in-order PE queue their
            # inputs have long been ready.
            pts = {}
            pvres = {}
            for h in range(H):
                pts[h] = scores_phase(h)
                if h - 1 >= 0:
                    pvres[h - 1] = pv_phase(h - 1, pts.pop(h - 1))
                if h - 2 >= 0:
                    epi_phase(h - 2, *pvres.pop(h - 2))
            epi_phase(H - 2, *pvres.pop(H - 2))  # before PV(15): fills a PE bubble
            pvres[H - 1] = pv_phase(H - 1, pts.pop(H - 1))
            epi_phase(H - 1, *pvres.pop(H - 1))

        x2 = x_sb  # attention residual is written in place

        if taps is not None:
            nc.sync.dma_start(taps["yT"], yT)

        # ---------------- phase 4: attention out-proj + residual ----------------
        ln2_stats = []
        with tc.tile_pool(name="psA2", bufs=4, space="PSUM") as psA2:
            for ti in range(NT):
                po = [psA2.tile([128, 512], F32, tag="pswo", name="pswo")
                      for _ in range(2)]
                for cp in range(NCP):
                    lt = yT[:, 2 * cp:2 * cp + 2, ti * 128:(ti + 1) * 128]
                    for nsp in range(2):
                        nc.tensor.matmul(
                            po[nsp], lhsT=lt,
                            rhs=wo_sb[:, cp, :, nsp * 512:(nsp + 1) * 512],
                            start=(cp == 0), stop=False, perf_mode=DR,
                        )
                for nsp in range(2):
                    nc.tensor.matmul(po[nsp], lhsT=ones_sb[0:1, 0:128],
                                     rhs=bo_sb[0:1, nsp * 512:(nsp + 1) * 512],
                                     start=False, stop=True)
                    nc.vector.tensor_add(
                        x2[:, ti, nsp * 512:(nsp + 1) * 512], po[nsp],
                        x_sb[:, ti, nsp * 512:(nsp + 1) * 512],
                    )
                # LN2 stats for this tile ride in the wo phase's engine slack
                ln2_stats.append(_ln_stats(nc, ln_small, x2, eps_sb, ti))

        est_attn.close()  # free kT/qT2/v/wo space before MLP tensors
        # ---------------- phase 5/6: LN2 + FC(gelu) ----------------
        # The MLP runs mixed precision: the last C/4 of the FC contraction and
        # the last 2C of the Wp contraction are fp8 DoubleRow (more would
        # breach the 2e-2 rel-err gate). All MLP weights are pre-scaled by 32
        # (exact power of two) so the fp8 rows sit in e4m3 normal range; the
        # PSUM epilogues compensate with scale=1/32.
        mlp_pool = est.enter_context(tc.tile_pool(name="mlpp", bufs=1))
        mT = mlp_pool.tile([128, KN8, T], BF16, tag="mT", name="mT")
        mT8 = mlp_pool.tile([128, 4 * NCK - KN8, T], F8, tag="mT8", name="mT8")
        outT = mlp_pool.tile([128, NCK, T], BF16, tag="outT", name="outT")
        with tc.tile_pool(name="h2Tp", bufs=1) as h2T_pool, \
             tc.tile_pool(name="psT2", bufs=2, space="PSUM") as psT2, \
             tc.tile_pool(name="psA3", bufs=4, space="PSUM") as psA3:
            h2T = h2T_pool.tile([128, NCK8, T], BF16, tag="h2T", name="h2T")
            h2T8 = h2T_pool.tile([128, 2, T], F8, tag="h2T8", name="h2T8")

            def h2_dest(cp, i):
                tsl = slice(i * 128, (i + 1) * 128)
                if 2 * cp < NCK8:
                    return h2T[:, 2 * cp:2 * cp + 2, tsl]
                return h2T8[:, 0:2, tsl]

            for i in range(NT):
                _ln_finish(nc, (ln_small, psT2), x2, h2_dest, ident_sb, i,
                           *ln2_stats[i])
            # Wp output-group 0 weights: prefetch during FC so the Wp phase
            # doesn't start with a weight-DMA stall.
            wp_pre = []
            for kn in range(8):
                wt = wp_pool.tile([128, 256], BF16, tag="wp", name="wpt")
                nc.sync.dma_start(wt, wp_d[kn * 128:(kn + 1) * 128, 0:256])
                wp_pre.append(wt)
            for fg in range(8):  # 512-wide feature groups over 4C
                if fg == 0:
                    wts = wf_fg0
                else:
                    wts = []
                    for ck in range(NCK8):
                        wt = wf_pool.tile([128, 512], BF16, tag="wfc", name="wfct")
                        nc.sync.dma_start(wt, wfc_d[ck * 128:(ck + 1) * 128, fg * 512:(fg + 1) * 512])
                        wts.append(wt)
                wt8 = wf_pool.tile([128, 2, 512], F8, tag="wfc8", bufs=3, name="wfc8t")
                nc.sync.dma_start(wt8, wfc8_d[:, 0, :, fg * 512:(fg + 1) * 512])
                for fl in range(4):
                    fn = fg * 4 + fl
                    pf = [psA3.tile([128, 512], F32, tag="psfc", name="psfc")
                          for _ in range(2)]
                    for ck in range(NCK8):
                        lt = wts[ck][:, fl * 128:(fl + 1) * 128]
                        for tsp in range(2):
                            nc.tensor.matmul(
                                pf[tsp], lhsT=lt,
                                rhs=h2T[:, ck, tsp * 512:(tsp + 1) * 512],
                                start=(ck == 0), stop=False,
                            )
                    lt8 = wt8[:, :, fl * 128:(fl + 1) * 128]
                    for tsp in range(2):
                        nc.tensor.matmul(
                            pf[tsp], lhsT=lt8,
                            rhs=h2T8[:, 0:2, tsp * 512:(tsp + 1) * 512],
                            start=False, stop=True, perf_mode=DR,
                        )
                    for tsp in range(2):
                        dst, dn = (mT, fn) if fn < KN8 else (mT8, fn - KN8)
                        nc.scalar.activation(
                            dst[:, dn, tsp * 512:(tsp + 1) * 512], pf[tsp],
                            AF.Gelu_apprx_tanh, bias=bfc_sb[:, fn:fn + 1], scale=1.0 / WS,
                        )

        # ---------------- phase 7: Wp (feature-major out) ----------------
        with tc.tile_pool(name="psW", bufs=4, space="PSUM") as psW, \
             tc.tile_pool(name="psT3", bufs=2, space="PSUM") as psT3, \
             tc.tile_pool(name="outp", bufs=8) as out_pool:

            def out_quarter(cg, outts):
                # transpose-back + residual + store for output column quarter
                # `cg` (the two outT chunks just produced); emitted mid-Wp so
                # the PE transposes run while the array is dense and warm, and
                # each 1MB output DMA overlaps the remaining Wp compute.
                for ti in range(NT):
                    if cg == 0:
                        outts.append(out_pool.tile([128, C], F32, tag="osb", name="outt"))
                    outt = outts[ti]
                    ps2 = psT3.tile([128, 2, 128], BF16, tag="pst3", name="ps2")
                    for cl in range(2):
                        cj = cg * 2 + cl
                        nc.tensor.transpose(ps2[:, cl, :], outT[:, cj, ti * 128:(ti + 1) * 128], ident_sb)
                    nc.vector.tensor_add(
                        outt[:, cg * 256:(cg + 1) * 256].rearrange("p (a b) -> p a b", a=2),
                        ps2,
                        x2[:, ti, cg * 256:(cg + 1) * 256].rearrange("p (a b) -> p a b", a=2),
                    )
                    nc.sync.dma_start(out_v[:, ti, cg * 256:(cg + 1) * 256],
                                      outt[:, cg * 256:(cg + 1) * 256])

            outts = []
            for cg in range(4):  # output feature groups of 256
                pss = [[psW.tile([128, 512], F32, tag="pswp", name="pswp")
                        for _ in range(2)] for _ in range(2)]
                for kn in range(KN8):  # bf16 contraction chunks
                    if cg == 0 and kn < 8:
                        wt = wp_pre[kn]
                    else:
                        wt = wp_pool.tile([128, 256], BF16, tag="wp", name="wpt")
                        nc.scalar.dma_start(wt, wp_d[kn * 128:(kn + 1) * 128, cg * 256:(cg + 1) * 256])
                    for cl in range(2):
                        for tsp in range(2):
                            nc.tensor.matmul(
                                pss[cl][tsp], lhsT=wt[:, cl * 128:(cl + 1) * 128],
                                rhs=mT[:, kn, tsp * 512:(tsp + 1) * 512],
                                start=(kn == 0), stop=False,
                            )
                for kp in range((4 * NCK - KN8) // 2):  # fp8 DoubleRow pair chunks
                    wt8 = wp_pool.tile([128, 2, 256], F8, tag="wp8", bufs=6, name="wp8t")
                    nc.scalar.dma_start(wt8, wp8_d[kp, :, :, cg * 256:(cg + 1) * 256])
                    for cl in range(2):
                        for tsp in range(2):
                            nc.tensor.matmul(
                                pss[cl][tsp], lhsT=wt8[:, :, cl * 128:(cl + 1) * 128],
                                rhs=mT8[:, 2 * kp:2 * kp + 2, tsp * 512:(tsp + 1) * 512],
                                start=False, stop=(kp == (4 * NCK - KN8) // 2 - 1), perf_mode=DR,
                            )
                for cl in range(2):
                    cn = cg * 2 + cl
                    for tsp in range(2):
                        nc.scalar.activation(
                            outT[:, cn, tsp * 512:(tsp + 1) * 512], pss[cl][tsp],
                            AF.Identity, bias=bp_sb[:, cn:cn + 1], scale=1.0 / WS,
                        )
                out_quarter(cg, outts)


def build_module(debug_taps=False):
    nc = bacc.Bacc("TRN2", target_bir_lowering=False, debug=False)

    def din(name, shape, dtype):
        return nc.dram_tensor(name, list(shape), dtype, kind="ExternalInput").ap()

    taps = None
    if debug_taps:
        taps = {
            "sums": nc.dram_tensor("dbg_sums", [H, T], F32, kind="ExternalOutput").ap(),
            "recips": nc.dram_tensor("dbg_recips", [H, T], F32, kind="ExternalOutput").ap(),
            "rbsrow": nc.dram_tensor("dbg_rbsrow", [H, T], F32, kind="ExternalOutput").ap(),
            "yT": nc.dram_tensor("dbg_yT", [128, NCK, T], F8, kind="ExternalOutput").ap(),
        }

    io = (
        din("x", (T, C), F32),
        din("wqk", (NCP, 128, 2, 2 * C), F8),
        din("wv", (128, NCP, 2, C), F8),
        din("bqk", (2 * C,), F32),
        din("wo", (128, NCP, 2, C), F8),
        din("bo", (C,), BF16),
        din("wfc", (C - 256, 4 * C), BF16),
        din("wfc8", (128, 1, 2, 4 * C), F8),
        din("bfc", (4 * C,), F32),
        din("wp", (C, C), BF16),
        din("wp8", (3 * NCP, 128, 2, C), F8),
        din("bp", (C,), F32),
        din("ident", (128, 128), BF16),
        din("maskt", (128, 128), BF16),
        nc.dram_tensor("out", [T, C], F32, kind="ExternalOutput").ap(),
    )
    with tile.TileContext(nc) as tc:
        _build_body(tc, io, taps=taps)
    nc.compile()
    return nc


def host_prepare(inputs):
    """Fold LN affine params / v-bias into weights; cast matmul weights to
    bf16 (MLP) / pair-interleaved fp8e4 (attention projections)."""
    bf = ml_dtypes.bfloat16
    f8 = ml_dtypes.float8_e4m3
    x = np.asarray(inputs["x"], np.float32)
    Wqkv = np.asarray(inputs["Wqkv"], np.float64)
    bqkv = np.asarray(inputs["bqkv"], np.float64)
    Wo = np.asarray(inputs["Wo"], np.float64)
    bo = np.asarray(inputs["bo"], np.float64)
    ln1_w = np.asarray(inputs["ln1_w"], np.float64)
    ln1_b = np.asarray(inputs["ln1_b"], np.float64)
    ln2_w = np.asarray(inputs["ln2_w"], np.float64)
    ln2_b = np.asarray(inputs["ln2_b"], np.float64)
    Wfc = np.asarray(inputs["Wfc"], np.float64)
    bfc = np.asarray(inputs["bfc"], np.float64)
    Wp = np.asarray(inputs["Wp"], np.float64)
    bp = np.asarray(inputs["bp"], np.float64)

    Wqkv_f = ln1_w[:, None] * Wqkv
    bqkv_f = bqkv + ln1_b @ Wqkv
    bo_f = bo + bqkv_f[2 * C:] @ Wo
    Wfc_f = ln2_w[:, None] * Wfc
    bfc_f = bfc + ln2_b @ Wfc

    def pair_cf(w):  # [K, F] -> [K/256, 128, 2, F]  (DoubleRow lhsT layout)
        return np.ascontiguousarray(
            w.reshape(-1, 2, 128, w.shape[1]).transpose(0, 2, 1, 3))

    def pair_pf(w):  # [K, F] -> [128, K/256, 2, F]  (DoubleRow rhs layout)
        return np.ascontiguousarray(
            w.reshape(-1, 2, 128, w.shape[1]).transpose(2, 0, 1, 3))

    common = {
        "wqk": pair_cf(Wqkv_f[:, :2 * C] * WS).astype(f8),
        "wv": pair_pf(Wqkv_f[:, 2 * C:] * WS).astype(f8),
        "bqk": bqkv_f[:2 * C].astype(np.float32),
        "wo": pair_pf(Wo).astype(f8),
        "bo": bo_f.astype(bf),
        # MLP weights are scaled by 32 (exact power of 2; epilogues divide it
        # back out) so their fp8 tail rows sit in e4m3 normal range.
        "wfc": (Wfc_f[:C - 256] * WS).astype(bf),
        "wfc8": pair_pf(Wfc_f[C - 256:] * WS).astype(f8),
        "bfc": bfc_f.astype(np.float32),
        "wp": (Wp[:C] * WS).astype(bf),
        "wp8": pair_cf(Wp[C:] * WS).astype(f8),
        "bp": bp.astype(np.float32),
        "ident": np.eye(128, dtype=bf),
        # additive causal mask, applied as lhsT in a PE accumulate vs ident:
        # ss[k, q] += maskt.T[k, q] = -1280 where k > q; exp(-1280/32) -> ~0
        "maskt": (np.triu(np.ones((128, 128)), k=1) * -1280.0).astype(bf),
    }
    return x, common


_NC_CACHE = None


def get_module():
    global _NC_CACHE
    if _NC_CACHE is None:
        _NC_CACHE = build_module()
    return _NC_CACHE


def run_with_results(inputs, **run_kwargs):
    x, common = host_prepare(inputs)
    nc = get_module()
    in_maps = [dict(common, x=np.ascontiguousarray(x[b])) for b in range(B)]
    res = run_bass_kernel_spmd(nc, in_maps, core_ids=list(range(N_CORES)), **run_kwargs)
    out = np.stack([res.results[b]["out"] for b in range(B)]).astype(np.float32)
    return out, res


def kernel(**inputs):
    return run_with_results(inputs)[0]
